# revision 6
# baseline (speedup 1.0000x reference)
"""Trainium2 Bass kernel for nn_Encoder_Attentioner (sparse_attention).

Mathematical collapse: the reference builds aff = Q @ K^T (8192x8192) but only
consumes per-key-batch block sums, which are linear:
    s[bq, i, bk] = q[bq,i] . sum_j k[bk, j]
Folding the q/k/conv 1x1 layers through that sum, the whole computation per
query batch b reduces to
    z[i] = x_b[i] . w_b,        w_b = A d_b + bias_chain
    A    = (conv_w^T + I) q_w^T (s0 k_w (conv_w + I))
    d_b  = column-sum of x over all *other* batches' rows
    out  = sigmoid(((z - zmin)/(zmax - zmin) - 0.65) / 0.15)

Distribution over 8 NeuronCores (two SPMD launches, host exchanges 2KB/core
between them — pure data movement):
  Launch 1 (k1): core b loads xT_b (channel-major, 1MB bf16) plus all four
  512x512 weight matrices. It computes its own colsum s_b (free-axis reduce)
  and, exploiting linearity of the chain, t_b = A s_b + bias_chain/7. The
  4-matvec chain hides under the weight DMA stream (each step fires as its
  matrix arrives); only step 4 trails the stream end.
  Launch 2 (k2): core b loads xT_b again plus the 7 other cores' t vectors;
  w_b = sum of those 7 (the bias/7 folding makes the sum exact), then the
  z row-dot in (128, 8) pixel layout, a partition_all_reduce-based min/max,
  and the fused scale/bias sigmoid. The z matmuls consume xT chunks as they
  arrive, so only the last chunk's 8 matmuls trail the stream.

Cost-model-aware choices: one DMA queue (sync) issues the stream in
consumption order so the exclusive DMA engines never idle; xT is split in two
so the colsum overlaps the weight transfers; min/max uses a single gpsimd
partition_all_reduce (max of [z, -z] reductions) so no PE transpose / PSUM
round-trip sits on the tail; the sigmoid activation table is pre-warmed under
the stream; biases join the PSUM accumulations as K=1 matmuls.

Host-side prep is layout only: dtype casts, transposes, folding the residual
identities into the weight matrices, and elementwise constant scaling of one
weight matrix and the two bias vectors.
"""

import numpy as np
import ml_dtypes

import concourse.bass as bass
import concourse.bacc as bacc
import concourse.mybir as mybir
import concourse.tile as tile
from concourse.bass_utils import run_bass_kernel_spmd

B, HW, C = 8, 1024, 512
P = 128
NCH = C // P   # 4 column chunks of 128 channels
NT = HW // P   # 8 pixel tiles of 128
N_CORES = 8
BF16 = mybir.dt.bfloat16
F32 = mybir.dt.float32
ATT_SCALE = float(1.0 / np.sqrt(C))                 # 1/sqrt(512)
BIAS_MULT = float((B - 1) * HW)                     # 7168
SCALE0 = ATT_SCALE / BIAS_MULT
THR_SCALE = 1.0 / 0.15
THR_BIAS = -0.65 / 0.15

_k1 = None
_k2 = None
last_results = {}


def _run_spmd(nc, in_maps, core_ids, attempts=3):
    """run_bass_kernel_spmd with retries for transient device errors."""
    import time

    last_err = None
    for i in range(attempts):
        try:
            return run_bass_kernel_spmd(nc, in_maps, core_ids)
        except Exception as e:  # noqa: BLE001 - transient PJRT/NRT failures
            last_err = e
            time.sleep(2.0 * (i + 1))
    raise last_err


def _new_nc():
    return bacc.Bacc(
        "TRN2",
        target_bir_lowering=False,
        debug=False,
        enable_asserts=False,
        num_devices=N_CORES,
    )


def _build_k1():
    """Per core: xT_b + weights -> t_b = A s_b + bias_chain/7, (128, 4) f32."""
    nc = _new_nc()
    xT = nc.dram_tensor("xT", [C, HW], BF16, kind="ExternalInput")
    # [0]: 7168*conv_b/7; [1]: scale*k_b/7
    cbt = nc.dram_tensor("cbt", [1, 2, NCH, P], F32, kind="ExternalInput")
    r1 = nc.dram_tensor("r1", [C, C], BF16, kind="ExternalInput")   # (conv_w^T+I)
    kwt = nc.dram_tensor("k_wt", [C, C], BF16, kind="ExternalInput")  # s0*k_w^T
    qw = nc.dram_tensor("q_w", [C, C], BF16, kind="ExternalInput")
    r2 = nc.dram_tensor("r2", [C, C], BF16, kind="ExternalInput")   # conv_w+I
    tvec = nc.dram_tensor("tvec", [P, NCH], F32, kind="ExternalOutput")

    madd = mybir.AluOpType.add
    X = mybir.AxisListType.X

    with tile.TileContext(nc) as tc:
        with (
            tc.tile_pool(name="sb", bufs=1) as sb,
            tc.tile_pool(name="psmv", bufs=2, space=bass.MemorySpace.PSUM) as psmv,
        ):
            # --- single-queue DMA stream in consumption order:
            # cbt (tiny), xT halves (colsum feeds step 1), then r1..r2.
            cbt_t = sb.tile([1, 2, NCH, P], F32, tag="cbt")
            nc.sync.dma_start(cbt_t[:], cbt.ap())

            xT_t = sb.tile([P, NCH, HW], BF16, tag="xT")
            xT_r = xT.ap().rearrange("(t p) i -> p t i", p=P)
            half = NCH // 2
            nc.sync.dma_start(xT_t[:, 0:half, :], xT_r[:, 0:half, :])
            nc.sync.dma_start(xT_t[:, half:NCH, :], xT_r[:, half:NCH, :])

            wt = {}
            for name, h in (("r1", r1), ("kwt", kwt), ("qw", qw), ("r2", r2)):
                t = sb.tile([P, NCH, C], BF16, tag=name)
                nc.sync.dma_start(t[:], h.ap().rearrange("(t p) c -> p t c", p=P))
                wt[name] = t

            ones1 = sb.tile([1, 1], F32)
            nc.gpsimd.memset(ones1[:], 1.0)

            # colsum s_b[c] = sum_hw xT[c, hw], chunk by chunk as halves land.
            s_f32 = sb.tile([P, NCH], F32, tag="s_f32")
            for cc in range(NCH):
                nc.vector.tensor_reduce(
                    s_f32[:, cc : cc + 1], xT_t[:, cc, :], axis=X, op=madd
                )
            s_bf = sb.tile([P, NCH], BF16, tag="s_bf")
            nc.vector.tensor_copy(s_bf[:], s_f32[:])

            def matvec_T(tiles, vec_bf, out_t, bias_row=None):
                """out (128,4) = R^T @ vec (+ bias row as K=1 matmul)."""
                mps = psmv.tile([P, NCH], F32, tag="mv")
                for oc in range(NCH):
                    for ic in range(NCH):
                        nc.tensor.matmul(
                            mps[:, oc : oc + 1],
                            tiles[:, ic, oc * P : (oc + 1) * P],
                            vec_bf[:, ic : ic + 1],
                            start=(ic == 0),
                            stop=(ic == NCH - 1 and bias_row is None),
                        )
                    if bias_row is not None:
                        nc.tensor.matmul(
                            mps[:, oc : oc + 1],
                            cbt_t[0:1, bias_row, oc, :],
                            ones1[:],
                            start=False,
                            stop=True,
                        )
                nc.vector.tensor_copy(out_t[:], mps[:])

            # t_b = (conv_w^T+I) q_w^T (s0 k_w (conv_w+I) s_b + biases/7)
            xr_bf = sb.tile([P, NCH], BF16, tag="xr_bf")
            matvec_T(wt["r1"], s_bf, xr_bf, bias_row=0)
            v_bf = sb.tile([P, NCH], BF16, tag="v_bf")
            matvec_T(wt["kwt"], xr_bf, v_bf, bias_row=1)
            u_bf = sb.tile([P, NCH], BF16, tag="u_bf")
            matvec_T(wt["qw"], v_bf, u_bf)
            res = sb.tile([P, NCH], F32, tag="res")
            matvec_T(wt["r2"], u_bf, res)
            nc.sync.dma_start(tvec.ap(), res[:])

    nc.compile()
    return nc


def _build_k2():
    """Per core: w = sum of 7 t-vectors; z = xT^T w; minmax; sigmoid."""
    nc = _new_nc()
    tso = nc.dram_tensor("tso", [B - 1, NCH, P], F32, kind="ExternalInput")
    xT = nc.dram_tensor("xT", [C, HW], BF16, kind="ExternalInput")
    out = nc.dram_tensor("out", [P, NT], F32, kind="ExternalOutput")

    mmin = mybir.AluOpType.min
    mmax = mybir.AluOpType.max
    madd = mybir.AluOpType.add
    mmul = mybir.AluOpType.mult
    X = mybir.AxisListType.X
    import concourse.bass_isa as bass_isa

    with tile.TileContext(nc) as tc:
        with (
            tc.tile_pool(name="sb", bufs=1) as sb,
            tc.tile_pool(name="ps", bufs=1, space=bass.MemorySpace.PSUM) as ps,
            tc.tile_pool(name="psw", bufs=1, space=bass.MemorySpace.PSUM) as psw,
        ):
            # stream: tso (tiny) then xT chunk by chunk (z consumes in order)
            tso_t = sb.tile([B - 1, NCH, P], F32, tag="tso")
            nc.sync.dma_start(tso_t[:], tso.ap())
            xT_t = sb.tile([P, NCH, HW], BF16, tag="xT")
            xT_r = xT.ap().rearrange("(t p) i -> p t i", p=P)
            for cc in range(NCH):
                nc.sync.dma_start(xT_t[:, cc, :], xT_r[:, cc, :])

            ones7 = sb.tile([B - 1, 1], F32)
            nc.gpsimd.memset(ones7[:], 1.0)
            # Warm the Sigmoid activation table off the critical path.
            warm = sb.tile([1, 2], F32, tag="warm")
            nc.gpsimd.memset(warm[:], 0.0)
            nc.scalar.activation(
                warm[:, 0:1],
                warm[:, 0:1],
                mybir.ActivationFunctionType.Sigmoid,
                bias=warm[:, 1:2],
                scale=warm[:, 1:2],
            )

            # w_b = sum of the 7 other cores' t vectors
            wps = psw.tile([P, NCH], F32, tag="w")
            for cc in range(NCH):
                nc.tensor.matmul(
                    wps[:, cc : cc + 1],
                    tso_t[:, cc, :],
                    ones7[:],
                    start=True,
                    stop=True,
                )
            w_bf = sb.tile([P, NCH], BF16, tag="w_bf")
            nc.vector.tensor_copy(w_bf[:], wps[:])

            # z in (128, 8) pixel layout; each column is one contiguous PSUM
            # accumulation group (interleaving groups breaks accumulation).
            zps = ps.tile([P, NT], F32, tag="z")
            for t in range(NT):
                for cc in range(NCH):
                    nc.tensor.matmul(
                        zps[:, t : t + 1],
                        xT_t[:, cc, t * P : (t + 1) * P],
                        w_bf[:, cc : cc + 1],
                        start=(cc == 0),
                        stop=(cc == NCH - 1),
                    )

            # Global min/max: per-partition [max, -min], one gpsimd
            # partition_all_reduce(max) replicates the global values to all
            # partitions, then the scale/bias for the fused sigmoid are a
            # few (128,1) DVE ops — no PE transpose on the tail.
            mm = sb.tile([P, 2], F32, tag="mm")
            nc.vector.tensor_reduce(mm[:, 0:1], zps[:], axis=X, op=mmax)
            nc.vector.tensor_reduce(mm[:, 1:2], zps[:], axis=X, op=mmin, negate=True)
            par = sb.tile([P, 2], F32, tag="par")
            nc.gpsimd.partition_all_reduce(
                par[:], mm[:], channels=P, reduce_op=bass_isa.ReduceOp.max
            )
            # rng = gmax - gmin = par0 + par1;  scale = (1/0.15)/rng
            # bias  = par1 * scale + THR_BIAS
            rng = sb.tile([P, 1], F32, tag="rng")
            nc.vector.tensor_tensor(rng[:], par[:, 0:1], par[:, 1:2], op=madd)
            rcp = sb.tile([P, 1], F32, tag="rcp")
            nc.vector.reciprocal_approx_fast(rcp[:], rng[:])
            bcs = sb.tile([P, 2], F32, tag="bcs")
            nc.vector.tensor_scalar(
                bcs[:, 0:1], rcp[:], THR_SCALE, 0.0, mmul, madd
            )
            t2 = sb.tile([P, 1], F32, tag="t2")
            nc.vector.tensor_tensor(t2[:], par[:, 1:2], bcs[:, 0:1], op=mmul)
            nc.vector.tensor_scalar(
                bcs[:, 1:2], t2[:], 1.0, THR_BIAS, mmul, madd
            )

            res = sb.tile([P, NT], F32, tag="res")
            nc.scalar.activation(
                res[:],
                zps[:],
                mybir.ActivationFunctionType.Sigmoid,
                bias=bcs[:, 1:2],
                scale=bcs[:, 0:1],
            )
            nc.sync.dma_start(out.ap(), res[:])

    nc.compile()
    return nc


def kernel(x, conv_w, conv_b, q_w, q_b, k_w, k_b):
    global _k1, _k2
    x = np.asarray(x, dtype=np.float32)
    conv_w = np.asarray(conv_w, dtype=np.float32)
    conv_b = np.asarray(conv_b, dtype=np.float32)
    q_w = np.asarray(q_w, dtype=np.float32)
    k_w = np.asarray(k_w, dtype=np.float32)
    k_b = np.asarray(k_b, dtype=np.float32)

    xf = x.reshape(B, HW, C)
    x_bf = xf.astype(ml_dtypes.bfloat16)
    xT_np = np.ascontiguousarray(np.transpose(x_bf, (0, 2, 1)))  # (B, C, HW)
    core_ids = list(range(N_CORES))

    eye = np.eye(C, dtype=np.float32)
    r1_np = np.ascontiguousarray(conv_w.T + eye).astype(ml_dtypes.bfloat16)
    r2_np = np.ascontiguousarray(conv_w + eye).astype(ml_dtypes.bfloat16)
    kwt_np = np.ascontiguousarray(k_w.T * np.float32(SCALE0)).astype(ml_dtypes.bfloat16)
    qw_np = np.ascontiguousarray(q_w).astype(ml_dtypes.bfloat16)
    inv7 = np.float32(1.0 / (B - 1))
    cbt_np = np.ascontiguousarray(np.stack([
        (conv_b * np.float32(BIAS_MULT) * inv7).reshape(NCH, P),
        (k_b * np.float32(ATT_SCALE) * inv7).reshape(NCH, P),
    ])[None])  # (1, 2, 4, 128)

    if _k1 is None:
        _k1 = _build_k1()
    in1 = [
        {
            "xT": xT_np[b],
            "cbt": cbt_np,
            "r1": r1_np,
            "k_wt": kwt_np,
            "q_w": qw_np,
            "r2": r2_np,
        }
        for b in range(B)
    ]
    r1_res = _run_spmd(_k1, in1, core_ids)
    last_results["k1"] = r1_res
    # per-batch t vectors, (128, 4) channel-chunked
    traw = [
        np.asarray(r1_res.results[b]["tvec"], dtype=np.float32).reshape(P, NCH)
        for b in range(B)
    ]

    if _k2 is None:
        _k2 = _build_k2()
    in2 = []
    for b in range(B):
        others = np.stack([traw[bb] for bb in range(B) if bb != b])  # (7, 128, 4)
        tso_np = np.ascontiguousarray(np.transpose(others, (0, 2, 1)))  # (7, 4, 128)
        in2.append({"tso": tso_np, "xT": xT_np[b]})
    r2_res = _run_spmd(_k2, in2, core_ids)
    last_results["k2"] = r2_res
    # out tile[p, t] = result pixel t*128+p  ->  (HW,) per batch
    outs = []
    for b in range(B):
        arr = np.asarray(r2_res.results[b]["out"], dtype=np.float32).reshape(P, NT)
        outs.append(arr.T.reshape(HW))
    return np.stack(outs).astype(np.float32)
